# revision 1
# baseline (speedup 1.0000x reference)
"""DynamicKBasis TRN2 kernel, v2 (3-joint groups, merged drains).

Per core (Bc=4096, 17 joints, groups of 3):
  L1 both nets: bf16x3-split matmul K=12, row-packed 3 joints/group.
  Hidden drains: one [128, 3*F] relu op per net per chunk (ACT=kw, DVE=basis).
  L2: fp32 matmuls M=18 at col-groups 32g, kwlog+par accumulate per joint block.
  LG drain (+b2) -> SBUF; PE transpose to batch-major; junk-skip drain to BM;
  batch-major epilogue; contiguous output DMAs; host reassembly.
"""
import contextlib
import dataclasses
import os
import time
import numpy as np
import ml_dtypes
import concourse.bass as bass
import concourse.tile as tile
from concourse import bacc, mybir
from concourse.bass_utils import run_bass_kernel_spmd

F32 = mybir.dt.float32
BF16 = mybir.dt.bfloat16
I32 = mybir.dt.int32
AL = mybir.AluOpType
AF = mybir.ActivationFunctionType

N_CORES = 8
B, NJ = 32768, 17
BC = B // N_CORES           # 4096
NG = 6                      # groups of <=3 joints
GJ = 3                      # joints per group
NSLOT = NG * GJ             # 18 slots (17 valid)
NCHUNK = 8
F = 512
NST = 32                    # 128-batch subtiles
SLOTW = GJ * 18             # 54 BM cols per (subtile, group)
STW = NG * SLOTW            # 324 BM cols per subtile


def rep(t_ap, dims, offset=0):
    return dataclasses.replace(t_ap, ap=[t_ap.ap[0]] + [list(x) for x in dims], offset=offset)


def bf16_split3(a):
    a = np.asarray(a, np.float32)
    hi = a.astype(ml_dtypes.bfloat16)
    r1 = a - hi.astype(np.float32)
    mid = r1.astype(ml_dtypes.bfloat16)
    lo = (r1 - mid.astype(np.float32)).astype(ml_dtypes.bfloat16)
    return hi, mid, lo


def host_prep(inputs):
    pp = np.asarray(inputs["pred_pts"], np.float32)
    gv = lambda k: np.asarray(inputs[k], np.float32)
    kW1, kb1, kW2, kb2 = gv("kW1"), gv("kb1"), gv("kW2"), gv("kb2")
    wW1, wb1, wW2, wb2 = gv("wW1"), gv("wb1"), gv("wW2"), gv("wb2")
    bW1, bb1, bW2, bb2 = gv("bW1"), gv("bb1"), gv("bW2"), gv("bb2")
    bias_zero = all(np.all(x == 0) for x in (kb1, wb1, bb1))

    kwW1 = np.concatenate([kW1, wW1], axis=2)               # [17, 2, 128]

    def w1_pack(w1):
        hi, mid, lo = bf16_split3(w1)
        out = np.zeros((NG, GJ, 12, w1.shape[2]), ml_dtypes.bfloat16)
        for p in range(NG):
            for g in range(GJ):
                j = GJ * p + g
                if j >= NJ:
                    continue
                out[p, g] = np.stack([
                    hi[j, 0], hi[j, 1],    # xhi . Whi
                    mid[j, 0], mid[j, 1],  # xhi . Wm
                    lo[j, 0], lo[j, 1],    # xhi . Wl
                    hi[j, 0], hi[j, 1],    # xm  . Whi
                    mid[j, 0], mid[j, 1],  # xm  . Wm
                    hi[j, 0], hi[j, 1],    # xl  . Whi
                ])
        return out

    W1KW = w1_pack(kwW1)
    W1B = w1_pack(bW1)

    W2KW = np.zeros((128, NJ * 18), np.float32)
    W2B = np.zeros((128, NJ * 18), np.float32)
    for j in range(NJ):
        W2KW[0:64, j * 18 + 0:j * 18 + 3] = kW2[j]
        W2KW[64:128, j * 18 + 3:j * 18 + 6] = wW2[j]
        W2B[:, j * 18 + 6:j * 18 + 18] = bW2[j]

    B1KW = np.zeros((128, NSLOT), np.float32)
    B1B = np.zeros((128, NSLOT), np.float32)
    B2 = np.zeros((NG, 128, 1), np.float32)
    for j in range(NJ):
        B1KW[0:64, j] = kb1[j]
        B1KW[64:128, j] = wb1[j]
        B1B[:, j] = bb1[j]
        p, g = j // GJ, j % GJ
        B2[p, 32 * g + 0:32 * g + 3, 0] = kb2[j]
        B2[p, 32 * g + 3:32 * g + 6, 0] = wb2[j]
        B2[p, 32 * g + 6:32 * g + 18, 0] = bb2[j]
    IDN = np.eye(128, dtype=np.float32)

    shared = dict(W1KW=W1KW, W1B=W1B, W2KW=W2KW, W2B=W2B, B1KW=B1KW, B1B=B1B,
                  B2=B2, IDN=IDN, SEED=np.zeros((1, 4), np.float32))

    hi, mid, lo = bf16_split3(pp)
    per_core = []
    for core in range(N_CORES):
        sl = slice(core * BC, (core + 1) * BC)
        xh = np.ascontiguousarray(hi[sl].transpose(1, 2, 0))
        xm = np.ascontiguousarray(mid[sl].transpose(1, 2, 0))
        xl = np.ascontiguousarray(lo[sl].transpose(1, 2, 0))
        XS = np.zeros((NG, GJ, 12, BC), ml_dtypes.bfloat16)
        for p in range(NG):
            for g in range(GJ):
                j = GJ * p + g
                if j >= NJ:
                    continue
                XS[p, g, 0:2] = xh[j]
                XS[p, g, 2:4] = xh[j]
                XS[p, g, 4:6] = xh[j]
                XS[p, g, 6:8] = xm[j]
                XS[p, g, 8:10] = xm[j]
                XS[p, g, 10:12] = xl[j]
        per_core.append(dict(XS=XS, **shared))
    return per_core, bias_zero


def build_program(reps=1, bias_zero=True):
    nc = bacc.Bacc("TRN2", target_bir_lowering=False, debug=False, num_devices=N_CORES)
    d = {}
    d["XS"] = nc.dram_tensor("XS", [NG, GJ, 12, BC], BF16, kind="ExternalInput").ap()
    d["W1KW"] = nc.dram_tensor("W1KW", [NG, GJ, 12, 128], BF16, kind="ExternalInput").ap()
    d["W1B"] = nc.dram_tensor("W1B", [NG, GJ, 12, 128], BF16, kind="ExternalInput").ap()
    d["W2KW"] = nc.dram_tensor("W2KW", [128, NJ * 18], F32, kind="ExternalInput").ap()
    d["W2B"] = nc.dram_tensor("W2B", [128, NJ * 18], F32, kind="ExternalInput").ap()
    d["B1KW"] = nc.dram_tensor("B1KW", [128, NSLOT], F32, kind="ExternalInput").ap()
    d["B1B"] = nc.dram_tensor("B1B", [128, NSLOT], F32, kind="ExternalInput").ap()
    d["B2"] = nc.dram_tensor("B2", [NG, 128, 1], F32, kind="ExternalInput").ap()
    d["IDN"] = nc.dram_tensor("IDN", [128, 128], F32, kind="ExternalInput").ap()
    d["SEED"] = nc.dram_tensor("SEED", [1, 4], F32, kind="ExternalInput").ap()

    o = {}
    o["W"] = nc.dram_tensor("W", [128, NST, NSLOT, 3], F32, kind="ExternalOutput").ap()
    o["KJS"] = nc.dram_tensor("KJS", [128, NST, NSLOT], I32, kind="ExternalOutput").ap()
    o["TOPK"] = nc.dram_tensor("TOPK", [128, NST, NSLOT, 3], I32, kind="ExternalOutput").ap()
    o["MU"] = nc.dram_tensor("MU", [128, NST, NSLOT, 6], F32, kind="ExternalOutput").ap()
    o["SIG"] = nc.dram_tensor("SIG", [128, NST, NSLOT, 6], F32, kind="ExternalOutput").ap()

    with tile.TileContext(nc) as tc:
        secs = os.environ.get("KSECS", "ABCDL")
        for _ in range(reps):
            build_kernel(tc, nc, d, o, secs, bias_zero)
    nc.compile()
    return nc


def build_kernel(tc, nc, d, o, secs="ABCDL", bias_zero=True):
    ctx = contextlib.ExitStack()
    with ctx:
        wpool = ctx.enter_context(tc.tile_pool(name="wts", bufs=1))
        xtp = ctx.enter_context(tc.tile_pool(name="xt", bufs=2))
        hkwp = ctx.enter_context(tc.tile_pool(name="hkw", bufs=2))
        hbp = ctx.enter_context(tc.tile_pool(name="hb", bufs=2))
        lgsbp = ctx.enter_context(tc.tile_pool(name="lgsb", bufs=2))
        bmp = ctx.enter_context(tc.tile_pool(name="bm", bufs=1))
        epp = ctx.enter_context(tc.tile_pool(name="ep", bufs=1))
        outp = ctx.enter_context(tc.tile_pool(name="outs", bufs=1))
        psKW = ctx.enter_context(tc.tile_pool(name="psKW", bufs=1, space=bass.MemorySpace.PSUM))
        psB = ctx.enter_context(tc.tile_pool(name="psB", bufs=1, space=bass.MemorySpace.PSUM))
        aux = ctx.enter_context(tc.tile_pool(name="aux", bufs=2, space=bass.MemorySpace.PSUM))

        # --- weights ---
        w1kw_t, w1b_t = [], []
        for p in range(NG):
            t1 = wpool.tile([128, 128], BF16, tag=f"w1kw{p}", name=f"w1kw{p}")
            t2 = wpool.tile([128, 128], BF16, tag=f"w1b{p}", name=f"w1b{p}")
            for g in range(GJ):
                if GJ * p + g >= NJ:
                    continue
                nc.sync.dma_start(t1[32 * g:32 * g + 12, :], d["W1KW"][p, g])
                nc.sync.dma_start(t2[32 * g:32 * g + 12, :], d["W1B"][p, g])
            w1kw_t.append(t1)
            w1b_t.append(t2)
        w2kw_t = wpool.tile([128, NJ * 18], F32, tag="w2kw")
        nc.sync.dma_start(w2kw_t[:], d["W2KW"][:])
        w2b_t = wpool.tile([128, NJ * 18], F32, tag="w2b")
        nc.sync.dma_start(w2b_t[:], d["W2B"][:])
        b1kw_t = wpool.tile([128, NSLOT], F32, tag="b1kw")
        nc.sync.dma_start(b1kw_t[:], d["B1KW"][:])
        b1b_t = wpool.tile([128, NSLOT], F32, tag="b1b")
        nc.sync.dma_start(b1b_t[:], d["B1B"][:])
        b2_t = wpool.tile([128, NG], F32, tag="b2")
        nc.sync.dma_start(b2_t[:], d["B2"].rearrange("p r c -> r (p c)"))
        idn_t = wpool.tile([128, 128], F32, tag="idn")
        nc.sync.dma_start(idn_t[:], d["IDN"][:])
        seed_t = wpool.tile([1, 4], F32, tag="seed")
        nc.sync.dma_start(seed_t[:], d["SEED"][:])

        bm_t = bmp.tile([128, NST * STW], F32, tag="bm")

        # --- main loop ---
        for p in range(NG):
            glist = [g for g in range(GJ) if GJ * p + g < NJ]
            xt = xtp.tile([128, BC], BF16, tag="xt")
            for g in glist:
                nc.sync.dma_start(xt[32 * g:32 * g + 12, :], d["XS"][p, g])
            lgsb = lgsbp.tile([128, BC], F32, tag="lgsb")
            if "A" in secs:
                for c in range(NCHUNK):
                    cs = slice(c * F, (c + 1) * F)
                    hpsk = psKW.tile([128, GJ, F], F32, tag="hpsk")
                    for g in glist:
                        nc.tensor.matmul(hpsk[:, g, :], w1kw_t[p][32 * g:32 * g + 12, :],
                                         xt[32 * g:32 * g + 12, cs], tile_position=(32 * g, 0))
                    hpsb = psB.tile([128, GJ, F], F32, tag="hpsb")
                    for g in glist:
                        nc.tensor.matmul(hpsb[:, g, :], w1b_t[p][32 * g:32 * g + 12, :],
                                         xt[32 * g:32 * g + 12, cs], tile_position=(32 * g, 0))
                    hkw = hkwp.tile([128, GJ, F], F32, tag="hkw")
                    hb = hbp.tile([128, GJ, F], F32, tag="hb")
                    if "D" in secs:
                        if bias_zero:
                            nc.scalar.activation(hkw[:], hpsk[:], AF.Relu)
                            nc.vector.tensor_scalar(hb[:], hpsb[:], 0.0, None, op0=AL.max)
                        else:
                            for g in glist:
                                j = GJ * p + g
                                nc.scalar.activation(hkw[:, g, :], hpsk[:, g, :], AF.Relu,
                                                     bias=b1kw_t[:, j:j + 1], scale=1.0)
                                nc.vector.tensor_scalar(hb[:, g, :], hpsb[:, g, :],
                                                        b1b_t[:, j:j + 1], 0.0,
                                                        op0=AL.add, op1=AL.max)
                    dolg = ("L" in secs) and ("D" in secs)
                    lg = aux.tile([128, F], F32, tag="lg")
                    for g in (glist if dolg else []):
                        j = GJ * p + g
                        nc.tensor.matmul(lg[32 * g:32 * g + 18, :],
                                         w2kw_t[:, j * 18:(j + 1) * 18], hkw[:, g, :],
                                         start=True, stop=False, tile_position=(0, 32 * g))
                    for g in (glist if dolg else []):
                        j = GJ * p + g
                        nc.tensor.matmul(lg[32 * g:32 * g + 18, :],
                                         w2b_t[:, j * 18:(j + 1) * 18], hb[:, g, :],
                                         start=False, stop=True, tile_position=(0, 32 * g))
                    if dolg:
                        if c % 2 == 0:
                            nc.vector.tensor_scalar(lgsb[:, cs], lg[:], b2_t[:, p:p + 1], None, op0=AL.add)
                        else:
                            nc.scalar.activation(lgsb[:, cs], lg[:], AF.Identity,
                                                 bias=b2_t[:, p:p + 1], scale=1.0)
            # transposes
            for s4 in (range(NCHUNK) if "B" in secs else []):
                tp = aux.tile([128, 4, 128], F32, tag="lg")
                for k in range(4):
                    s = 4 * s4 + k
                    nc.tensor.transpose(tp[:, k, :], lgsb[:, s * 128:(s + 1) * 128], idn_t[:])
                tpap = tp[:]
                inv = rep(tpap, [[128, 4], [32, GJ], [1, 18]])
                outv = rep(bm_t[:], [[STW, 4], [18, GJ], [1, 18]],
                           offset=s4 * 4 * STW + SLOTW * p)
                if s4 % 2 == 0:
                    nc.vector.tensor_copy(outv, inv)
                else:
                    nc.scalar.copy(outv, inv)

        # --- epilogue ---
        if "C" not in secs:
            return
        bmap = bm_t[:]
        SJ = [(STW, NST), (SLOTW, NG), (18, GJ)]

        def bmv(off, *extra):
            return rep(bmap, SJ + list(extra), offset=off)

        e_t = epp.tile([128, NST * NSLOT * 3], F32, tag="e")
        s_t = epp.tile([128, NST * NSLOT], F32, tag="s")
        r_t = epp.tile([128, NST * NSLOT], F32, tag="r")
        w_t = outp.tile([128, NST * NSLOT * 3], F32, tag="w")
        kjs_t = outp.tile([128, NST * NSLOT], I32, tag="kjs")
        topk_t = outp.tile([128, NST * NSLOT * 3], I32, tag="topk")
        mu_t = outp.tile([128, NST * NSLOT * 6], F32, tag="mu")
        sig_t = outp.tile([128, NST * NSLOT * 6], F32, tag="sig")
        sc = []
        for i in range(6):
            sct = epp.tile([128, NST * NSLOT], F32, tag=f"sc{i}", name=f"sc{i}")
            sc.append(sct)

        W3 = NSLOT * 3
        ein = bmv(3, (1, 3))
        eout = rep(e_t[:], [[W3, NST], [3, NSLOT], [1, 3]])
        nc.scalar.activation(eout, ein, AF.Exp)

        def ev(k):
            return rep(e_t[:], [[W3, NST], [3, NSLOT]], offset=k)

        def sv(t, off=0):
            return rep(t[:], [[NSLOT, NST], [1, NSLOT]], offset=off)

        nc.vector.tensor_tensor(sv(s_t), ev(0), ev(1), op=AL.add)
        nc.vector.tensor_tensor(sv(s_t), sv(s_t), ev(2), op=AL.add)
        nc.vector.reciprocal(r_t[:], s_t[:])
        ein3 = rep(e_t[:], [[W3, NST], [3, NSLOT], [1, 3]])
        rin3 = rep(r_t[:], [[NSLOT, NST], [1, NSLOT], [0, 3]])
        wout3 = rep(w_t[:], [[W3, NST], [3, NSLOT], [1, 3]])
        nc.vector.tensor_tensor(wout3, ein3, rin3, op=AL.mult)

        kl = lambda k: bmv(k)
        c_, mx, dd, tt_, uu, vv = (sv(x) for x in sc)
        nc.vector.tensor_tensor(c_, kl(1), kl(0), op=AL.is_gt)
        nc.vector.tensor_tensor(mx, kl(0), kl(1), op=AL.max)
        nc.vector.tensor_tensor(dd, kl(2), mx, op=AL.is_gt)
        nc.vector.tensor_scalar(tt_, c_, -1.0, 2.0, op0=AL.mult, op1=AL.add)
        nc.vector.tensor_tensor(uu, dd, tt_, op=AL.mult)
        nc.vector.tensor_tensor(vv, uu, c_, op=AL.add)
        kjsv = rep(kjs_t[:], [[NSLOT, NST], [1, NSLOT]])
        nc.vector.tensor_scalar(kjsv, vv, 1.0, None, op0=AL.add)

        wv = lambda k: rep(w_t[:], [[W3, NST], [3, NSLOT]], offset=k)
        c01, c02, c12, p0, p1, p2 = (sv(x) for x in sc)
        nc.vector.tensor_tensor(c01, wv(0), wv(1), op=AL.is_ge)
        nc.vector.tensor_tensor(c02, wv(0), wv(2), op=AL.is_ge)
        nc.vector.tensor_tensor(c12, wv(1), wv(2), op=AL.is_ge)
        nc.vector.tensor_scalar(p0, c01, -1.0, 2.0, op0=AL.mult, op1=AL.add)
        nc.vector.tensor_tensor(p0, p0, c02, op=AL.subtract)
        nc.vector.tensor_scalar(p1, c12, -1.0, 1.0, op0=AL.mult, op1=AL.add)
        nc.vector.tensor_tensor(p1, p1, c01, op=AL.add)
        nc.vector.tensor_tensor(p2, c02, c12, op=AL.add)
        e1, e2 = sv(sc[0]), sv(sc[1])
        for pos in range(3):
            nc.vector.tensor_scalar(e1, p1, float(pos), None, op0=AL.is_equal)
            nc.vector.tensor_scalar(e2, p2, float(pos), None, op0=AL.is_equal)
            tkv = rep(topk_t[:], [[W3, NST], [3, NSLOT]], offset=pos)
            nc.vector.scalar_tensor_tensor(tkv, e2, 2.0, e1, op0=AL.mult, op1=AL.add)

        for dd_ in range(2):
            muin = bmv(6 + dd_, (4, 3))
            muout = rep(mu_t[:], [[NSLOT * 6, NST], [6, NSLOT], [2, 3]], offset=dd_)
            nc.scalar.copy(muout, muin)
            sgin = bmv(8 + dd_, (4, 3))
            sgout = rep(sig_t[:], [[NSLOT * 6, NST], [6, NSLOT], [2, 3]], offset=dd_)
            nc.scalar.activation(sgout, sgin, AF.Exp)

        nc.sync.dma_start(o["W"].rearrange("r a b c -> r (a b c)"), w_t[:])
        nc.sync.dma_start(o["KJS"].rearrange("r a b -> r (a b)"), kjs_t[:])
        nc.sync.dma_start(o["TOPK"].rearrange("r a b c -> r (a b c)"), topk_t[:])
        nc.sync.dma_start(o["MU"].rearrange("r a b c -> r (a b c)"), mu_t[:])
        nc.sync.dma_start(o["SIG"].rearrange("r a b c -> r (a b c)"), sig_t[:])


def bench(nc, per_core, iters=30):
    import jax
    from jax.sharding import Mesh, PartitionSpec
    from jax.experimental.shard_map import shard_map
    from concourse import bass2jax, mybir as _mb
    bass2jax.install_neuronx_cc_hook()
    in_names, out_names, out_avals, zero_outs = [], [], [], []
    partition_name = nc.partition_id_tensor.name if nc.partition_id_tensor else None
    for alloc in nc.m.functions[0].allocations:
        if not isinstance(alloc, _mb.MemoryLocationSet):
            continue
        name = alloc.memorylocations[0].name
        if alloc.kind == "ExternalInput":
            if name != partition_name:
                in_names.append(name)
        elif alloc.kind == "ExternalOutput":
            shape = tuple(alloc.tensor_shape)
            dtype = _mb.dt.np(alloc.dtype)
            out_names.append(name)
            out_avals.append(jax.core.ShapedArray(shape, dtype))
            zero_outs.append(np.zeros(shape, dtype))
    n_params = len(in_names)
    all_in_names = list(in_names) + list(out_names)
    if partition_name is not None:
        all_in_names.append(partition_name)

    def _body(*args):
        operands = list(args)
        if partition_name is not None:
            operands.append(bass2jax.partition_id_tensor())
        outs = bass2jax._bass_exec_p.bind(
            *operands, out_avals=tuple(out_avals), in_names=tuple(all_in_names),
            out_names=tuple(out_names), lowering_input_output_aliases=(),
            sim_require_finite=True, sim_require_nnan=True, nc=nc)
        return tuple(outs)

    devices = jax.devices()[:N_CORES]
    mesh = Mesh(np.asarray(devices), ("core",))
    in_specs = (PartitionSpec("core"),) * (n_params + len(out_names))
    out_specs = (PartitionSpec("core"),) * len(out_names)
    fn = jax.jit(shard_map(_body, mesh=mesh, in_specs=in_specs, out_specs=out_specs,
                           check_rep=False), keep_unused=True)
    concat_in = [np.concatenate([np.asarray(per_core[c][i_name]) for c in range(N_CORES)], axis=0)
                 for i_name in in_names]
    concat_zeros = [np.zeros((N_CORES * z.shape[0], *z.shape[1:]), z.dtype) for z in zero_outs]
    sharding = jax.sharding.NamedSharding(mesh, PartitionSpec("core"))
    dev_in = [jax.device_put(a, sharding) for a in concat_in + concat_zeros]
    if os.environ.get("KRUNNER"):
        def unit(M):
            for _m in range(M):
                outs = fn(*dev_in)
            jax.block_until_ready(outs)
        return unit, None
    outs = fn(*dev_in)
    jax.block_until_ready(outs)
    times = []
    for _ in range(iters):
        t0 = time.perf_counter()
        outs = fn(*dev_in)
        jax.block_until_ready(outs)
        times.append(time.perf_counter() - t0)
    times_ns = sorted(int(t * 1e9) for t in times)
    results = [
        {name: np.asarray(outs[i]).reshape(N_CORES, *out_avals[i].shape)[c]
         for i, name in enumerate(out_names)}
        for c in range(N_CORES)
    ]
    return times_ns[0], results


def assemble(results):
    w = np.zeros((B, NJ, 3), np.float32)
    kjs = np.zeros((B, NJ), np.int32)
    topk = np.zeros((B, NJ, 3), np.int32)
    mu = np.zeros((B, NJ, 3, 2), np.float32)
    sig = np.zeros((B, NJ, 3, 2), np.float32)
    for core, r in enumerate(results):
        sl = slice(core * BC, (core + 1) * BC)

        def fix(a):
            return a.transpose(1, 0, *range(2, a.ndim)).reshape(BC, *a.shape[2:])

        w[sl] = fix(r["W"])[:, :NJ, :]
        kjs[sl] = fix(r["KJS"])[:, :NJ]
        topk[sl] = fix(r["TOPK"])[:, :NJ, :]
        mu[sl] = fix(r["MU"])[:, :NJ, :].reshape(BC, NJ, 3, 2)
        sig[sl] = fix(r["SIG"])[:, :NJ, :].reshape(BC, NJ, 3, 2)
    return w, mu, sig, kjs, topk


_PROG_CACHE = {}


def kernel(**inputs):
    per_core, bias_zero = host_prep(inputs)
    nc = _PROG_CACHE.get(bias_zero)
    if nc is None:
        nc = build_program(1, bias_zero)
        _PROG_CACHE[bias_zero] = nc
    res = run_bass_kernel_spmd(nc, per_core, core_ids=list(range(N_CORES)))
    return assemble(res.results)




# revision 4
# speedup vs baseline: 1.3800x; 1.3800x over previous
"""DynamicKBasis TRN2 kernel, v2 (3-joint groups, merged drains).

Per core (Bc=4096, 17 joints, groups of 3):
  L1 both nets: bf16x3-split matmul K=12, row-packed 3 joints/group.
  Hidden drains: one [128, 3*F] relu op per net per chunk (ACT=kw, DVE=basis).
  L2: fp32 matmuls M=18 at col-groups 32g, kwlog+par accumulate per joint block.
  LG drain (+b2) -> SBUF; PE transpose to batch-major; junk-skip drain to BM;
  batch-major epilogue; contiguous output DMAs; host reassembly.
"""
import contextlib
import dataclasses
import os
import time
import numpy as np
import ml_dtypes
import concourse.bass as bass
import concourse.tile as tile
from concourse import bacc, mybir
from concourse.bass_utils import run_bass_kernel_spmd

F32 = mybir.dt.float32
BF16 = mybir.dt.bfloat16
I32 = mybir.dt.int32
AL = mybir.AluOpType
AF = mybir.ActivationFunctionType

N_CORES = 8
B, NJ = 32768, 17
BC = B // N_CORES           # 4096
NG = 6                      # groups of <=3 joints
GJ = 3                      # joints per group
NSLOT = NG * GJ             # 18 slots (17 valid)
NCHUNK = 8
F = 512
NST = 32                    # 128-batch subtiles
SLOTW = GJ * 18             # 54 BM cols per (subtile, group)
STW = NG * SLOTW            # 324 BM cols per subtile


def rep(t_ap, dims, offset=0):
    return dataclasses.replace(t_ap, ap=[t_ap.ap[0]] + [list(x) for x in dims], offset=offset)


def bf16_split3(a):
    a = np.asarray(a, np.float32)
    hi = a.astype(ml_dtypes.bfloat16)
    r1 = a - hi.astype(np.float32)
    mid = r1.astype(ml_dtypes.bfloat16)
    lo = (r1 - mid.astype(np.float32)).astype(ml_dtypes.bfloat16)
    return hi, mid, lo


def host_prep(inputs):
    pp = np.asarray(inputs["pred_pts"], np.float32)
    gv = lambda k: np.asarray(inputs[k], np.float32)
    kW1, kb1, kW2, kb2 = gv("kW1"), gv("kb1"), gv("kW2"), gv("kb2")
    wW1, wb1, wW2, wb2 = gv("wW1"), gv("wb1"), gv("wW2"), gv("wb2")
    bW1, bb1, bW2, bb2 = gv("bW1"), gv("bb1"), gv("bW2"), gv("bb2")
    bias_zero = all(np.all(x == 0) for x in (kb1, wb1, bb1))

    kwW1 = np.concatenate([kW1, wW1], axis=2)               # [17, 2, 128]

    def w1_pack(w1):
        hi, mid, lo = bf16_split3(w1)
        out = np.zeros((NG, GJ, 12, w1.shape[2]), ml_dtypes.bfloat16)
        for p in range(NG):
            for g in range(GJ):
                j = GJ * p + g
                if j >= NJ:
                    continue
                out[p, g] = np.stack([
                    hi[j, 0], hi[j, 1],    # xhi . Whi
                    mid[j, 0], mid[j, 1],  # xhi . Wm
                    lo[j, 0], lo[j, 1],    # xhi . Wl
                    hi[j, 0], hi[j, 1],    # xm  . Whi
                    mid[j, 0], mid[j, 1],  # xm  . Wm
                    hi[j, 0], hi[j, 1],    # xl  . Whi
                ])
        return out

    W1KW = w1_pack(kwW1)
    W1B = w1_pack(bW1)

    W2KW = np.zeros((128, NJ * 18), np.float32)
    W2B = np.zeros((128, NJ * 18), np.float32)
    for j in range(NJ):
        W2KW[0:64, j * 18 + 0:j * 18 + 3] = kW2[j]
        W2KW[64:128, j * 18 + 3:j * 18 + 6] = wW2[j]
        W2B[:, j * 18 + 6:j * 18 + 18] = bW2[j]

    B1KW = np.zeros((128, NSLOT), np.float32)
    B1B = np.zeros((128, NSLOT), np.float32)
    B2 = np.zeros((NG, 128, 1), np.float32)
    for j in range(NJ):
        B1KW[0:64, j] = kb1[j]
        B1KW[64:128, j] = wb1[j]
        B1B[:, j] = bb1[j]
        p, g = j // GJ, j % GJ
        B2[p, 32 * g + 0:32 * g + 3, 0] = kb2[j]
        B2[p, 32 * g + 3:32 * g + 6, 0] = wb2[j]
        B2[p, 32 * g + 6:32 * g + 18, 0] = bb2[j]
    IDN = np.eye(128, dtype=np.float32)

    shared = dict(W1KW=W1KW, W1B=W1B, W2KW=W2KW, W2B=W2B, B1KW=B1KW, B1B=B1B,
                  B2=B2, IDN=IDN, SEED=np.zeros((1, 4), np.float32))

    hi, mid, lo = bf16_split3(pp)
    per_core = []
    for core in range(N_CORES):
        sl = slice(core * BC, (core + 1) * BC)
        xh = np.ascontiguousarray(hi[sl].transpose(1, 2, 0))
        xm = np.ascontiguousarray(mid[sl].transpose(1, 2, 0))
        xl = np.ascontiguousarray(lo[sl].transpose(1, 2, 0))
        XS = np.zeros((NG, GJ, 12, BC), ml_dtypes.bfloat16)
        for p in range(NG):
            for g in range(GJ):
                j = GJ * p + g
                if j >= NJ:
                    continue
                XS[p, g, 0:2] = xh[j]
                XS[p, g, 2:4] = xh[j]
                XS[p, g, 4:6] = xh[j]
                XS[p, g, 6:8] = xm[j]
                XS[p, g, 8:10] = xm[j]
                XS[p, g, 10:12] = xl[j]
        per_core.append(dict(XS=XS, **shared))
    return per_core, bias_zero


def build_program(reps=1, bias_zero=True):
    nc = bacc.Bacc("TRN2", target_bir_lowering=False, debug=False, num_devices=N_CORES)
    d = {}
    d["XS"] = nc.dram_tensor("XS", [NG, GJ, 12, BC], BF16, kind="ExternalInput").ap()
    d["W1KW"] = nc.dram_tensor("W1KW", [NG, GJ, 12, 128], BF16, kind="ExternalInput").ap()
    d["W1B"] = nc.dram_tensor("W1B", [NG, GJ, 12, 128], BF16, kind="ExternalInput").ap()
    d["W2KW"] = nc.dram_tensor("W2KW", [128, NJ * 18], F32, kind="ExternalInput").ap()
    d["W2B"] = nc.dram_tensor("W2B", [128, NJ * 18], F32, kind="ExternalInput").ap()
    d["B1KW"] = nc.dram_tensor("B1KW", [128, NSLOT], F32, kind="ExternalInput").ap()
    d["B1B"] = nc.dram_tensor("B1B", [128, NSLOT], F32, kind="ExternalInput").ap()
    d["B2"] = nc.dram_tensor("B2", [NG, 128, 1], F32, kind="ExternalInput").ap()
    d["IDN"] = nc.dram_tensor("IDN", [128, 128], F32, kind="ExternalInput").ap()
    d["SEED"] = nc.dram_tensor("SEED", [1, 4], F32, kind="ExternalInput").ap()

    o = {}
    o["W"] = nc.dram_tensor("W", [128, NST, NSLOT, 3], F32, kind="ExternalOutput").ap()
    o["KJS"] = nc.dram_tensor("KJS", [128, NST, NSLOT], I32, kind="ExternalOutput").ap()
    o["TOPK"] = nc.dram_tensor("TOPK", [128, NST, NSLOT, 3], I32, kind="ExternalOutput").ap()
    o["MU"] = nc.dram_tensor("MU", [128, NST, NSLOT, 6], F32, kind="ExternalOutput").ap()
    o["SIG"] = nc.dram_tensor("SIG", [128, NST, NSLOT, 6], F32, kind="ExternalOutput").ap()

    with tile.TileContext(nc) as tc:
        secs = os.environ.get("KSECS", "ABCDL")
        for _ in range(reps):
            build_kernel(tc, nc, d, o, secs, bias_zero)
    nc.compile()
    return nc


def build_kernel(tc, nc, d, o, secs="ABCDL", bias_zero=True):
    ctx = contextlib.ExitStack()
    with ctx:
        wpool = ctx.enter_context(tc.tile_pool(name="wts", bufs=1))
        xtp = ctx.enter_context(tc.tile_pool(name="xt", bufs=2))
        hkwp = ctx.enter_context(tc.tile_pool(name="hkw", bufs=3))
        hbp = ctx.enter_context(tc.tile_pool(name="hb", bufs=3))
        lgsbp = ctx.enter_context(tc.tile_pool(name="lgsb", bufs=2))
        bmp = ctx.enter_context(tc.tile_pool(name="bm", bufs=1))
        epp = ctx.enter_context(tc.tile_pool(name="ep", bufs=1))
        outp = ctx.enter_context(tc.tile_pool(name="outs", bufs=1))
        psKW = ctx.enter_context(tc.tile_pool(name="psKW", bufs=1, space=bass.MemorySpace.PSUM))
        psB = ctx.enter_context(tc.tile_pool(name="psB", bufs=1, space=bass.MemorySpace.PSUM))
        aux = ctx.enter_context(tc.tile_pool(name="aux", bufs=2, space=bass.MemorySpace.PSUM))

        # --- weights ---
        w1kw_t, w1b_t = [], []
        for p in range(NG):
            t1 = wpool.tile([128, 128], BF16, tag=f"w1kw{p}", name=f"w1kw{p}")
            t2 = wpool.tile([128, 128], BF16, tag=f"w1b{p}", name=f"w1b{p}")
            for g in range(GJ):
                if GJ * p + g >= NJ:
                    continue
                nc.sync.dma_start(t1[32 * g:32 * g + 12, :], d["W1KW"][p, g])
                nc.sync.dma_start(t2[32 * g:32 * g + 12, :], d["W1B"][p, g])
            w1kw_t.append(t1)
            w1b_t.append(t2)
        w2kw_t = wpool.tile([128, NJ * 18], F32, tag="w2kw")
        nc.sync.dma_start(w2kw_t[:], d["W2KW"][:])
        w2b_t = wpool.tile([128, NJ * 18], F32, tag="w2b")
        nc.sync.dma_start(w2b_t[:], d["W2B"][:])
        b1kw_t = wpool.tile([128, NSLOT], F32, tag="b1kw")
        nc.sync.dma_start(b1kw_t[:], d["B1KW"][:])
        b1b_t = wpool.tile([128, NSLOT], F32, tag="b1b")
        nc.sync.dma_start(b1b_t[:], d["B1B"][:])
        b2_t = wpool.tile([128, NG], F32, tag="b2")
        nc.sync.dma_start(b2_t[:], d["B2"].rearrange("p r c -> r (p c)"))
        idn_t = wpool.tile([128, 128], F32, tag="idn")
        nc.sync.dma_start(idn_t[:], d["IDN"][:])
        seed_t = wpool.tile([1, 4], F32, tag="seed")
        nc.sync.dma_start(seed_t[:], d["SEED"][:])

        bm_t = bmp.tile([128, NST * STW], F32, tag="bm")

        # --- main loop ---
        for p in range(NG):
            glist = [g for g in range(GJ) if GJ * p + g < NJ]
            xt = xtp.tile([128, BC], BF16, tag="xt")
            for g in glist:
                nc.sync.dma_start(xt[32 * g:32 * g + 12, :], d["XS"][p, g])
            lgsb = lgsbp.tile([128, BC], F32, tag="lgsb")
            if "A" in secs:
                for c in range(NCHUNK):
                    cs = slice(c * F, (c + 1) * F)
                    hpsk = psKW.tile([128, GJ, F], F32, tag="hpsk")
                    for g in glist:
                        nc.tensor.matmul(hpsk[:, g, :], w1kw_t[p][32 * g:32 * g + 12, :],
                                         xt[32 * g:32 * g + 12, cs], tile_position=(32 * g, 0))
                    hpsb = psB.tile([128, GJ, F], F32, tag="hpsb")
                    for g in glist:
                        nc.tensor.matmul(hpsb[:, g, :], w1b_t[p][32 * g:32 * g + 12, :],
                                         xt[32 * g:32 * g + 12, cs], tile_position=(32 * g, 0))
                    hkw = hkwp.tile([128, GJ, F], F32, tag="hkw")
                    hb = hbp.tile([128, GJ, F], F32, tag="hb")
                    if "D" in secs:
                        if bias_zero:
                            nc.scalar.activation(hkw[:], hpsk[:], AF.Relu)
                            nc.vector.tensor_scalar(hb[:], hpsb[:], 0.0, None, op0=AL.max)
                        else:
                            for g in glist:
                                j = GJ * p + g
                                nc.scalar.activation(hkw[:, g, :], hpsk[:, g, :], AF.Relu,
                                                     bias=b1kw_t[:, j:j + 1], scale=1.0)
                                nc.vector.tensor_scalar(hb[:, g, :], hpsb[:, g, :],
                                                        b1b_t[:, j:j + 1], 0.0,
                                                        op0=AL.add, op1=AL.max)
                    dolg = ("L" in secs) and ("D" in secs)
                    lg = aux.tile([128, F], F32, tag="lg")
                    for g in (glist if dolg else []):
                        j = GJ * p + g
                        nc.tensor.matmul(lg[32 * g:32 * g + 18, :],
                                         w2kw_t[:, j * 18:(j + 1) * 18], hkw[:, g, :],
                                         start=True, stop=False, tile_position=(0, 32 * g))
                    for g in (glist if dolg else []):
                        j = GJ * p + g
                        nc.tensor.matmul(lg[32 * g:32 * g + 18, :],
                                         w2b_t[:, j * 18:(j + 1) * 18], hb[:, g, :],
                                         start=False, stop=True, tile_position=(0, 32 * g))
                    if dolg:
                        if c % 2 == 0:
                            nc.vector.tensor_scalar(lgsb[:, cs], lg[:], b2_t[:, p:p + 1], None, op0=AL.add)
                        else:
                            nc.scalar.activation(lgsb[:, cs], lg[:], AF.Identity,
                                                 bias=b2_t[:, p:p + 1], scale=1.0)
            # transposes
            for s4 in (range(NCHUNK) if "B" in secs else []):
                tp = aux.tile([128, 4, 128], F32, tag="lg")
                for k in range(4):
                    s = 4 * s4 + k
                    nc.tensor.transpose(tp[:, k, :], lgsb[:, s * 128:(s + 1) * 128], idn_t[:])
                tpap = tp[:]
                inv = rep(tpap, [[128, 4], [32, GJ], [1, 18]])
                outv = rep(bm_t[:], [[STW, 4], [18, GJ], [1, 18]],
                           offset=s4 * 4 * STW + SLOTW * p)
                if s4 % 2 == 0:
                    nc.vector.tensor_copy(outv, inv)
                else:
                    nc.scalar.copy(outv, inv)

        # --- epilogue ---
        if "C" not in secs:
            return
        bmap = bm_t[:]
        SJ = [(STW, NST), (SLOTW, NG), (18, GJ)]

        def bmv(off, *extra):
            return rep(bmap, SJ + list(extra), offset=off)

        e_t = epp.tile([128, NST * NSLOT * 3], F32, tag="e")
        s_t = epp.tile([128, NST * NSLOT], F32, tag="s")
        r_t = epp.tile([128, NST * NSLOT], F32, tag="r")
        w_t = outp.tile([128, NST * NSLOT * 3], F32, tag="w")
        kjs_t = outp.tile([128, NST * NSLOT], I32, tag="kjs")
        topk_t = outp.tile([128, NST * NSLOT * 3], I32, tag="topk")
        mu_t = outp.tile([128, NST * NSLOT * 6], F32, tag="mu")
        sig_t = outp.tile([128, NST * NSLOT * 6], F32, tag="sig")
        sc = []
        for i in range(6):
            sct = epp.tile([128, NST * NSLOT], F32, tag=f"sc{i}", name=f"sc{i}")
            sc.append(sct)

        W3 = NSLOT * 3
        ein = bmv(3, (1, 3))
        eout = rep(e_t[:], [[W3, NST], [3, NSLOT], [1, 3]])
        nc.scalar.activation(eout, ein, AF.Exp)

        def ev(k):
            return rep(e_t[:], [[W3, NST], [3, NSLOT]], offset=k)

        def sv(t, off=0):
            return rep(t[:], [[NSLOT, NST], [1, NSLOT]], offset=off)

        nc.vector.tensor_tensor(sv(s_t), ev(0), ev(1), op=AL.add)
        nc.vector.tensor_tensor(sv(s_t), sv(s_t), ev(2), op=AL.add)
        nc.vector.reciprocal(r_t[:], s_t[:])
        ein3 = rep(e_t[:], [[W3, NST], [3, NSLOT], [1, 3]])
        rin3 = rep(r_t[:], [[NSLOT, NST], [1, NSLOT], [0, 3]])
        wout3 = rep(w_t[:], [[W3, NST], [3, NSLOT], [1, 3]])
        nc.vector.tensor_tensor(wout3, ein3, rin3, op=AL.mult)

        kl = lambda k: bmv(k)
        c_, mx, dd, tt_, uu, vv = (sv(x) for x in sc)
        nc.vector.tensor_tensor(c_, kl(1), kl(0), op=AL.is_gt)
        nc.vector.tensor_tensor(mx, kl(0), kl(1), op=AL.max)
        nc.vector.tensor_tensor(dd, kl(2), mx, op=AL.is_gt)
        nc.vector.tensor_scalar(tt_, c_, -1.0, 2.0, op0=AL.mult, op1=AL.add)
        nc.vector.tensor_tensor(uu, dd, tt_, op=AL.mult)
        nc.vector.tensor_tensor(vv, uu, c_, op=AL.add)
        kjsv = rep(kjs_t[:], [[NSLOT, NST], [1, NSLOT]])
        nc.vector.tensor_scalar(kjsv, vv, 1.0, None, op0=AL.add)

        wv = lambda k: rep(w_t[:], [[W3, NST], [3, NSLOT]], offset=k)
        c01, c02, c12, p0, p1, p2 = (sv(x) for x in sc)
        nc.vector.tensor_tensor(c01, wv(0), wv(1), op=AL.is_ge)
        nc.vector.tensor_tensor(c02, wv(0), wv(2), op=AL.is_ge)
        nc.vector.tensor_tensor(c12, wv(1), wv(2), op=AL.is_ge)
        nc.vector.tensor_scalar(p0, c01, -1.0, 2.0, op0=AL.mult, op1=AL.add)
        nc.vector.tensor_tensor(p0, p0, c02, op=AL.subtract)
        nc.vector.tensor_scalar(p1, c12, -1.0, 1.0, op0=AL.mult, op1=AL.add)
        nc.vector.tensor_tensor(p1, p1, c01, op=AL.add)
        nc.vector.tensor_tensor(p2, c02, c12, op=AL.add)
        e1, e2 = sv(sc[0]), sv(sc[1])
        for pos in range(3):
            nc.vector.tensor_scalar(e1, p1, float(pos), None, op0=AL.is_equal)
            nc.vector.tensor_scalar(e2, p2, float(pos), None, op0=AL.is_equal)
            tkv = rep(topk_t[:], [[W3, NST], [3, NSLOT]], offset=pos)
            nc.vector.scalar_tensor_tensor(tkv, e2, 2.0, e1, op0=AL.mult, op1=AL.add)

        for dd_ in range(2):
            muin = bmv(6 + dd_, (4, 3))
            muout = rep(mu_t[:], [[NSLOT * 6, NST], [6, NSLOT], [2, 3]], offset=dd_)
            nc.scalar.copy(muout, muin)
            sgin = bmv(8 + dd_, (4, 3))
            sgout = rep(sig_t[:], [[NSLOT * 6, NST], [6, NSLOT], [2, 3]], offset=dd_)
            nc.scalar.activation(sgout, sgin, AF.Exp)

        nc.sync.dma_start(o["W"].rearrange("r a b c -> r (a b c)"), w_t[:])
        nc.sync.dma_start(o["KJS"].rearrange("r a b -> r (a b)"), kjs_t[:])
        nc.sync.dma_start(o["TOPK"].rearrange("r a b c -> r (a b c)"), topk_t[:])
        nc.sync.dma_start(o["MU"].rearrange("r a b c -> r (a b c)"), mu_t[:])
        nc.sync.dma_start(o["SIG"].rearrange("r a b c -> r (a b c)"), sig_t[:])


def bench(nc, per_core, iters=30):
    import jax
    from jax.sharding import Mesh, PartitionSpec
    from jax.experimental.shard_map import shard_map
    from concourse import bass2jax, mybir as _mb
    bass2jax.install_neuronx_cc_hook()
    in_names, out_names, out_avals, zero_outs = [], [], [], []
    partition_name = nc.partition_id_tensor.name if nc.partition_id_tensor else None
    for alloc in nc.m.functions[0].allocations:
        if not isinstance(alloc, _mb.MemoryLocationSet):
            continue
        name = alloc.memorylocations[0].name
        if alloc.kind == "ExternalInput":
            if name != partition_name:
                in_names.append(name)
        elif alloc.kind == "ExternalOutput":
            shape = tuple(alloc.tensor_shape)
            dtype = _mb.dt.np(alloc.dtype)
            out_names.append(name)
            out_avals.append(jax.core.ShapedArray(shape, dtype))
            zero_outs.append(np.zeros(shape, dtype))
    n_params = len(in_names)
    all_in_names = list(in_names) + list(out_names)
    if partition_name is not None:
        all_in_names.append(partition_name)

    def _body(*args):
        operands = list(args)
        if partition_name is not None:
            operands.append(bass2jax.partition_id_tensor())
        outs = bass2jax._bass_exec_p.bind(
            *operands, out_avals=tuple(out_avals), in_names=tuple(all_in_names),
            out_names=tuple(out_names), lowering_input_output_aliases=(),
            sim_require_finite=True, sim_require_nnan=True, nc=nc)
        return tuple(outs)

    devices = jax.devices()[:N_CORES]
    mesh = Mesh(np.asarray(devices), ("core",))
    in_specs = (PartitionSpec("core"),) * (n_params + len(out_names))
    out_specs = (PartitionSpec("core"),) * len(out_names)
    fn = jax.jit(shard_map(_body, mesh=mesh, in_specs=in_specs, out_specs=out_specs,
                           check_rep=False), keep_unused=True)
    concat_in = [np.concatenate([np.asarray(per_core[c][i_name]) for c in range(N_CORES)], axis=0)
                 for i_name in in_names]
    concat_zeros = [np.zeros((N_CORES * z.shape[0], *z.shape[1:]), z.dtype) for z in zero_outs]
    sharding = jax.sharding.NamedSharding(mesh, PartitionSpec("core"))
    dev_in = [jax.device_put(a, sharding) for a in concat_in + concat_zeros]
    if os.environ.get("KRUNNER"):
        def unit(M):
            for _m in range(M):
                outs = fn(*dev_in)
            jax.block_until_ready(outs)
        return unit, None
    outs = fn(*dev_in)
    jax.block_until_ready(outs)
    times = []
    for _ in range(iters):
        t0 = time.perf_counter()
        outs = fn(*dev_in)
        jax.block_until_ready(outs)
        times.append(time.perf_counter() - t0)
    times_ns = sorted(int(t * 1e9) for t in times)
    results = [
        {name: np.asarray(outs[i]).reshape(N_CORES, *out_avals[i].shape)[c]
         for i, name in enumerate(out_names)}
        for c in range(N_CORES)
    ]
    return times_ns[0], results


def assemble(results):
    w = np.zeros((B, NJ, 3), np.float32)
    kjs = np.zeros((B, NJ), np.int32)
    topk = np.zeros((B, NJ, 3), np.int32)
    mu = np.zeros((B, NJ, 3, 2), np.float32)
    sig = np.zeros((B, NJ, 3, 2), np.float32)
    for core, r in enumerate(results):
        sl = slice(core * BC, (core + 1) * BC)

        def fix(a):
            return a.transpose(1, 0, *range(2, a.ndim)).reshape(BC, *a.shape[2:])

        w[sl] = fix(r["W"])[:, :NJ, :]
        kjs[sl] = fix(r["KJS"])[:, :NJ]
        topk[sl] = fix(r["TOPK"])[:, :NJ, :]
        mu[sl] = fix(r["MU"])[:, :NJ, :].reshape(BC, NJ, 3, 2)
        sig[sl] = fix(r["SIG"])[:, :NJ, :].reshape(BC, NJ, 3, 2)
    return w, mu, sig, kjs, topk


_PROG_CACHE = {}


def kernel(**inputs):
    per_core, bias_zero = host_prep(inputs)
    nc = _PROG_CACHE.get(bias_zero)
    if nc is None:
        nc = build_program(1, bias_zero)
        _PROG_CACHE[bias_zero] = nc
    res = run_bass_kernel_spmd(nc, per_core, core_ids=list(range(N_CORES)))
    return assemble(res.results)


